# revision 12
# baseline (speedup 1.0000x reference)
"""Graph-ODE (GCN message passing) Trainium2 kernel.

Problem: h0 = x @ W_fc + b_fc; 4 Euler steps of
  h <- h + 0.25 * relu(gcn2(relu(gcn1(h)))),  gcn(h) = (adj @ h) @ W + b
with B=32, N=4096, IN_DIM=64, H=128.

Strategy (8 NeuronCores, data-parallel over batch):
 - Each core owns 4 batches; adj (pre-transposed + tiled on host) and
   weights are replicated. No collectives.
 - Aggregation adj @ V: stationary = adjT column-block tiles [m,128n],
   moving = V in node-major interleaved layout [m, (b,h)] (free dim 512 =
   4 batches x H), PSUM accumulates over 32 m-tiles.
 - Projection is fused with the layout transpose: PE-transpose of each
   agg tile gives aggT [h,n]; matmul(lhsT=aggT, rhs=W) yields z back in
   node-major layout. Bias (zero in this problem) is added with a K=1
   matmul of ones^T @ b in the bias-capable build variant.
 - Aggregation matmuls run in fp8-e4m3 with perf_mode=DoubleRow (256-K
   virtual rows, ~2x bf16 throughput). adj is scaled by 4096 on the host
   so its entries sit in e4m3 normal range; the scale is folded back via
   W/4096 in the projection, so no extra ops are spent on it. The 4096-
   term aggregation averages out the fp8 rounding noise. Projections,
   transposes, and the fc layer stay bf16 (fc as a 3-term hi/lo split),
   and the Euler state h stays fp32 in SBUF.
   Measured: ~1.11 ms HW exec, 8.5e-5 relative error vs fp32 reference
   (a bf16-aggregation build, fp8=False, runs ~1.99 ms at 1.1e-5).
"""
import sys

sys.path.insert(0, "/opt/trn_rl_repo")

import numpy as np
import ml_dtypes

import concourse.bass as bass
import concourse.mybir as mybir
import concourse.tile as tile
from concourse.bass_utils import run_bass_kernel_spmd

BF16 = mybir.dt.bfloat16
FP8 = mybir.dt.float8e4
F32 = mybir.dt.float32
ADJ_SCALE = 4096.0

B, N, IN_DIM, H = 32, 4096, 64, 128
N_CORES = 8
BL = B // N_CORES          # 4 batches per core
NT = N // 128              # 32 node tiles
FREE = BL * H              # 512 moving free dim
STEP = 0.25
N_STEPS = 4


def _split_multiwait(nc):
    """This walrus build accepts only ONE sync-wait command per engine
    instruction (incl. drains). Hoist extra waits onto preceding
    single-wait InstNoOps on the same engine."""
    import bass_rust
    for fn in nc.m.functions:
        for blk in fn.blocks:
            out = []
            for inst in blk.instructions:
                si = inst.sync_info
                if (si is not None and si.on_wait and len(si.on_wait) > 1
                        and type(inst).__name__ not in (
                            "InstTensorLoad", "InstTensorSave", "InstTrigger")):
                    waits = list(si.on_wait)
                    for w in waits[:-1]:
                        out.append(mybir.InstNoOp(
                            name=nc.get_next_instruction_name(),
                            engine=inst.engine, ins=[], outs=[],
                            sync_info=bass_rust.SyncInfo(
                                on_wait=[w], on_update=[]),
                        ))
                    inst.sync_info = bass_rust.SyncInfo(
                        on_wait=[waits[-1]], on_update=list(si.on_update))
                out.append(inst)
            blk.instructions = out


def _build(with_bias, fp8=True):
    nc = bass.Bass()

    adt = FP8 if fp8 else BF16
    adjt = nc.dram_tensor("adjt", [NT, 128, NT, 128], adt, kind="ExternalInput")
    xt_hi = nc.dram_tensor("xt_hi", [BL, IN_DIM, N], BF16, kind="ExternalInput")
    xt_lo = nc.dram_tensor("xt_lo", [BL, IN_DIM, N], BF16, kind="ExternalInput")
    wfc_hi = nc.dram_tensor("wfc_hi", [IN_DIM, H], BF16, kind="ExternalInput")
    wfc_lo = nc.dram_tensor("wfc_lo", [IN_DIM, H], BF16, kind="ExternalInput")
    w1 = nc.dram_tensor("w1", [H, H], BF16, kind="ExternalInput")
    w2 = nc.dram_tensor("w2", [H, H], BF16, kind="ExternalInput")
    if with_bias:
        b_fc = nc.dram_tensor("b_fc", [1, H], BF16, kind="ExternalInput")
        b1 = nc.dram_tensor("b1", [1, H], BF16, kind="ExternalInput")
        b2 = nc.dram_tensor("b2", [1, H], BF16, kind="ExternalInput")
        ones = nc.dram_tensor("ones", [1, H], BF16, kind="ExternalInput")
    ident = nc.dram_tensor("ident", [128, 128], BF16, kind="ExternalInput")
    out = nc.dram_tensor("out", [BL, N, H], F32, kind="ExternalOutput")

    relu = mybir.ActivationFunctionType.Relu
    XC = 2048  # phase-0 x chunk (columns)

    with tile.TileContext(nc) as tc:
        with tc.tile_pool(name="res", bufs=1) as res, \
             tc.tile_pool(name="wgt", bufs=1) as wgt, \
             tc.tile_pool(name="xs", bufs=2) as xs, \
             tc.tile_pool(name="adjs", bufs=3) as adjs, \
             tc.tile_pool(name="work", bufs=3) as work, \
             tc.tile_pool(name="ps", bufs=2, space="PSUM") as ps, \
             tc.tile_pool(name="psagg", bufs=3, space="PSUM") as psagg:

            # --- resident state: h (fp32) and bf16 activations, layout
            # [p, nt, b, h] (node-major interleaved)
            Hsb = res.tile([128, NT, BL, H], F32, tag="Hsb")
            Hbf = res.tile([128, NT, BL, H], adt, tag="Hbf")
            Tbf = res.tile([128, NT, BL, H], adt, tag="Tbf")

            # --- constants
            wfc_hi_t = wgt.tile([IN_DIM, H], BF16, tag="wfch")
            wfc_lo_t = wgt.tile([IN_DIM, H], BF16, tag="wfcl")
            w1_t = wgt.tile([H, H], BF16, tag="w1")
            w2_t = wgt.tile([H, H], BF16, tag="w2")
            id_t = wgt.tile([128, 128], BF16, tag="ident")
            nc.sync.dma_start(wfc_hi_t[:], wfc_hi[:])
            nc.sync.dma_start(wfc_lo_t[:], wfc_lo[:])
            nc.sync.dma_start(w1_t[:], w1[:])
            nc.sync.dma_start(w2_t[:], w2[:])
            nc.sync.dma_start(id_t[:], ident[:])
            if with_bias:
                bfc_t = wgt.tile([1, H], BF16, tag="bfc")
                b1_t = wgt.tile([1, H], BF16, tag="b1")
                b2_t = wgt.tile([1, H], BF16, tag="b2")
                ones_t = wgt.tile([1, H], BF16, tag="ones")
                nc.sync.dma_start(bfc_t[:], b_fc[:])
                nc.sync.dma_start(b1_t[:], b1[:])
                nc.sync.dma_start(b2_t[:], b2[:])
                nc.sync.dma_start(ones_t[:], ones[:])

            # --- phase 0: h0 = x @ W_fc + b_fc (3-term hi/lo split)
            for b in range(BL):
                for ch in range(N // XC):
                    xh = xs.tile([IN_DIM, XC], BF16, tag="xh")
                    xl = xs.tile([IN_DIM, XC], BF16, tag="xl")
                    nc.sync.dma_start(xh[:], xt_hi[b, :, bass.ts(ch, XC)])
                    nc.scalar.dma_start(xl[:], xt_lo[b, :, bass.ts(ch, XC)])
                    for j in range(XC // 128):
                        nt = ch * (XC // 128) + j
                        pz = ps.tile([128, H], F32, tag="pz")
                        xhs = xh[:, bass.ts(j, 128)]
                        xls = xl[:, bass.ts(j, 128)]
                        nc.tensor.matmul(pz[:], xhs, wfc_hi_t[:], start=True, stop=False)
                        nc.tensor.matmul(pz[:], xls, wfc_hi_t[:], start=False, stop=False)
                        last = not with_bias
                        nc.tensor.matmul(pz[:], xhs, wfc_lo_t[:], start=False, stop=last)
                        if with_bias:
                            nc.tensor.matmul(pz[:], ones_t[:], bfc_t[:],
                                             start=False, stop=True)
                        nc.vector.tensor_copy(Hsb[:, nt, b, :], pz[:])
                        nc.scalar.activation(Hbf[:, nt, b, :], pz[:],
                                             mybir.ActivationFunctionType.Copy)

            # --- 4 Euler steps x 2 GCN layers
            for step in range(N_STEPS):
                for layer in range(2):
                    V = Hbf if layer == 0 else Tbf
                    W = w1_t if layer == 0 else w2_t
                    bias = None if not with_bias else (b1_t if layer == 0 else b2_t)
                    for nt in range(NT):
                        blk = adjs.tile([128, NT, 128], adt, tag="adjblk")
                        nc.sync.dma_start(blk[:], adjt[nt])
                        pa = psagg.tile([128, BL, H], F32, tag="pagg")
                        if fp8:
                            for mt2 in range(NT // 2):
                                nc.tensor.matmul(
                                    pa[:], blk[:, bass.ts(mt2, 2), :],
                                    V[:, bass.ts(mt2, 2), :, :],
                                    start=(mt2 == 0), stop=(mt2 == NT // 2 - 1),
                                    perf_mode=mybir.MatmulPerfMode.DoubleRow)
                        else:
                            for mt in range(NT):
                                nc.tensor.matmul(pa[:], blk[:, mt, :], V[:, mt, :, :],
                                                 start=(mt == 0), stop=(mt == NT - 1))
                        agg = work.tile([128, BL, H], BF16, tag="agg")
                        nc.vector.tensor_copy(agg[:], pa[:])
                        # all 4 per-batch transposes into ONE psum bank tile,
                        # drained with one wide ACT copy
                        ptr = ps.tile([128, BL, 128], BF16, tag="ptr")
                        for b in range(BL):
                            nc.tensor.transpose(ptr[:, b, :], agg[:, b, :], id_t[:])
                        aggT = work.tile([128, BL, 128], BF16, tag="aggT")
                        nc.scalar.activation(aggT[:], ptr[:],
                                             mybir.ActivationFunctionType.Copy)
                        # 4 projections into ONE psum bank tile
                        pz = ps.tile([128, BL, H], F32, tag="pz")
                        for b in range(BL):
                            nc.tensor.matmul(pz[:, b, :], aggT[:, b, :], W[:],
                                             start=True, stop=bias is None)
                            if bias is not None:
                                nc.tensor.matmul(pz[:, b, :], ones_t[:], bias[:],
                                                 start=False, stop=True)
                        if layer == 0:
                            nc.scalar.activation(Tbf[:, nt, :, :], pz[:], relu)
                        else:
                            tmp = work.tile([128, BL, H], F32, tag="tmp")
                            nc.scalar.activation(tmp[:], pz[:], relu, scale=STEP)
                            nc.vector.tensor_add(Hsb[:, nt, :, :],
                                                 Hsb[:, nt, :, :], tmp[:])
                            if step == N_STEPS - 1:
                                # final h: stream out as soon as ready, on the
                                # gpsimd DMA queue so the adjT stream (sync
                                # queue) is not head-blocked
                                for b in range(BL):
                                    nc.gpsimd.dma_start(
                                        out[b, bass.ts(nt, 128), :],
                                        Hsb[:, nt, b, :])
                            else:
                                # refresh bf/fp8 copy of h in-loop (avoids a
                                # trailing cast pass at the step boundary)
                                nc.vector.tensor_copy(Hbf[:, nt, :, :],
                                                      Hsb[:, nt, :, :])

    _split_multiwait(nc)
    return nc


_NC_CACHE = {}


def _get_nc(with_bias, fp8=True):
    key = (with_bias, fp8)
    if key not in _NC_CACHE:
        _NC_CACHE[key] = _build(with_bias, fp8)
    return _NC_CACHE[key]


def _bf(a):
    return np.ascontiguousarray(a.astype(ml_dtypes.bfloat16))


def _prep_in_maps(x, adj, W_fc, b_fc, W1, b1, W2, b2, fp8=True):
    x = np.asarray(x, dtype=np.float32)
    adj = np.asarray(adj, dtype=np.float32)
    W_fc = np.asarray(W_fc, dtype=np.float32)
    b_fc = np.asarray(b_fc, dtype=np.float32)
    W1 = np.asarray(W1, dtype=np.float32)
    b1 = np.asarray(b1, dtype=np.float32)
    W2 = np.asarray(W2, dtype=np.float32)
    b2 = np.asarray(b2, dtype=np.float32)

    with_bias = bool(np.any(b_fc) or np.any(b1) or np.any(b2))

    # host layout prep (replicated operands)
    adjt = np.ascontiguousarray(
        adj.T.reshape(NT, 128, NT, 128).transpose(2, 1, 0, 3))  # [nt, p, mt, j]
    if fp8:
        adjt = np.ascontiguousarray((adjt * ADJ_SCALE).astype(ml_dtypes.float8_e4m3))
        w1h, w2h = _bf(W1 / ADJ_SCALE), _bf(W2 / ADJ_SCALE)
    else:
        adjt = _bf(adjt)
        w1h, w2h = _bf(W1), _bf(W2)
    wfc_hi = W_fc.astype(ml_dtypes.bfloat16)
    wfc_lo = _bf(W_fc - wfc_hi.astype(np.float32))
    wfc_hi = np.ascontiguousarray(wfc_hi)
    shared = {
        "adjt": adjt,
        "wfc_hi": wfc_hi, "wfc_lo": wfc_lo,
        "w1": w1h, "w2": w2h,
        "ident": np.eye(128, dtype=ml_dtypes.bfloat16),
    }
    if with_bias:
        shared.update({
            "b_fc": _bf(b_fc.reshape(1, H)),
            "b1": _bf(b1.reshape(1, H)),
            "b2": _bf(b2.reshape(1, H)),
            "ones": np.ones((1, H), dtype=ml_dtypes.bfloat16),
        })

    in_maps = []
    for c in range(N_CORES):
        xs = x[c * BL:(c + 1) * BL]                 # [BL, N, IN_DIM]
        xt = np.ascontiguousarray(xs.transpose(0, 2, 1))  # [BL, IN_DIM, N]
        xt_hi = xt.astype(ml_dtypes.bfloat16)
        xt_lo = _bf(xt - xt_hi.astype(np.float32))
        in_maps.append({**shared,
                        "xt_hi": np.ascontiguousarray(xt_hi),
                        "xt_lo": xt_lo})
    return in_maps, with_bias


FP8_DEFAULT = True


def kernel(**inputs):
    in_maps, with_bias = _prep_in_maps(**inputs, fp8=FP8_DEFAULT)
    nc = _get_nc(with_bias, FP8_DEFAULT)
    res = run_bass_kernel_spmd(nc, in_maps, core_ids=list(range(N_CORES)))
    return np.concatenate([res.results[c]["out"] for c in range(N_CORES)], axis=0)


def run_traced(**inputs):
    in_maps, with_bias = _prep_in_maps(**inputs, fp8=FP8_DEFAULT)
    nc = _get_nc(with_bias, FP8_DEFAULT)
    return run_bass_kernel_spmd(nc, in_maps, core_ids=list(range(N_CORES)),
                                trace=True)


# revision 14
# speedup vs baseline: 1.0009x; 1.0009x over previous
"""Graph-ODE (GCN message passing) Trainium2 kernel.

Problem: h0 = x @ W_fc + b_fc; 4 Euler steps of
  h <- h + 0.25 * relu(gcn2(relu(gcn1(h)))),  gcn(h) = (adj @ h) @ W + b
with B=32, N=4096, IN_DIM=64, H=128.

Strategy (8 NeuronCores, data-parallel over batch):
 - Each core owns 4 batches; adj (pre-transposed + tiled on host) and
   weights are replicated. No collectives.
 - Aggregation adj @ V: stationary = adjT column-block tiles [m,128n],
   moving = V in node-major interleaved layout [m, (b,h)] (free dim 512 =
   4 batches x H), PSUM accumulates over 32 m-tiles.
 - Projection is fused with the layout transpose: PE-transpose of each
   agg tile gives aggT [h,n]; matmul(lhsT=aggT, rhs=W) yields z back in
   node-major layout. Bias (zero in this problem) is added with a K=1
   matmul of ones^T @ b in the bias-capable build variant.
 - Aggregation matmuls run in fp8-e4m3 with perf_mode=DoubleRow (256-K
   virtual rows, ~2x bf16 throughput). adj is scaled by 4096 on the host
   so its entries sit in e4m3 normal range; the scale is folded back via
   W/4096 in the projection, so no extra ops are spent on it. The 4096-
   term aggregation averages out the fp8 rounding noise. Projections,
   transposes, and the fc layer stay bf16 (fc as a 3-term hi/lo split),
   and the Euler state h stays fp32 in SBUF.
   Measured: ~1.11 ms HW exec, 8.5e-5 relative error vs fp32 reference
   (a bf16-aggregation build, fp8=False, runs ~1.99 ms at 1.1e-5).
"""
import sys

sys.path.insert(0, "/opt/trn_rl_repo")

import numpy as np
import ml_dtypes

import concourse.bass as bass
import concourse.mybir as mybir
import concourse.tile as tile
from concourse.bass_utils import run_bass_kernel_spmd

BF16 = mybir.dt.bfloat16
FP8 = mybir.dt.float8e4
F32 = mybir.dt.float32
ADJ_SCALE = 4096.0

B, N, IN_DIM, H = 32, 4096, 64, 128
N_CORES = 8
BL = B // N_CORES          # 4 batches per core
NT = N // 128              # 32 node tiles
FREE = BL * H              # 512 moving free dim
STEP = 0.25
N_STEPS = 4


def _split_multiwait(nc):
    """This walrus build accepts only ONE sync-wait command per engine
    instruction (incl. drains). Hoist extra waits onto preceding
    single-wait InstNoOps on the same engine."""
    import bass_rust
    for fn in nc.m.functions:
        for blk in fn.blocks:
            out = []
            for inst in blk.instructions:
                si = inst.sync_info
                if (si is not None and si.on_wait and len(si.on_wait) > 1
                        and type(inst).__name__ not in (
                            "InstTensorLoad", "InstTensorSave", "InstTrigger")):
                    waits = list(si.on_wait)
                    for w in waits[:-1]:
                        out.append(mybir.InstNoOp(
                            name=nc.get_next_instruction_name(),
                            engine=inst.engine, ins=[], outs=[],
                            sync_info=bass_rust.SyncInfo(
                                on_wait=[w], on_update=[]),
                        ))
                    inst.sync_info = bass_rust.SyncInfo(
                        on_wait=[waits[-1]], on_update=list(si.on_update))
                out.append(inst)
            blk.instructions = out


def _build(with_bias, fp8=True):
    nc = bass.Bass()

    adt = FP8 if fp8 else BF16
    adjt = nc.dram_tensor("adjt", [NT, 128, NT, 128], adt, kind="ExternalInput")
    xt_hi = nc.dram_tensor("xt_hi", [BL, IN_DIM, N], BF16, kind="ExternalInput")
    xt_lo = nc.dram_tensor("xt_lo", [BL, IN_DIM, N], BF16, kind="ExternalInput")
    wfc_hi = nc.dram_tensor("wfc_hi", [IN_DIM, H], BF16, kind="ExternalInput")
    wfc_lo = nc.dram_tensor("wfc_lo", [IN_DIM, H], BF16, kind="ExternalInput")
    w1 = nc.dram_tensor("w1", [H, H], BF16, kind="ExternalInput")
    w2 = nc.dram_tensor("w2", [H, H], BF16, kind="ExternalInput")
    if with_bias:
        b_fc = nc.dram_tensor("b_fc", [1, H], BF16, kind="ExternalInput")
        b1 = nc.dram_tensor("b1", [1, H], BF16, kind="ExternalInput")
        b2 = nc.dram_tensor("b2", [1, H], BF16, kind="ExternalInput")
        ones = nc.dram_tensor("ones", [1, H], BF16, kind="ExternalInput")
    ident = nc.dram_tensor("ident", [128, 128], BF16, kind="ExternalInput")
    out = nc.dram_tensor("out", [BL, N, H], F32, kind="ExternalOutput")

    relu = mybir.ActivationFunctionType.Relu
    XC = 2048  # phase-0 x chunk (columns)

    with tile.TileContext(nc) as tc:
        with tc.tile_pool(name="res", bufs=1) as res, \
             tc.tile_pool(name="wgt", bufs=1) as wgt, \
             tc.tile_pool(name="xs", bufs=3) as xs, \
             tc.tile_pool(name="adjs", bufs=3) as adjs, \
             tc.tile_pool(name="work", bufs=3) as work, \
             tc.tile_pool(name="ps", bufs=2, space="PSUM") as ps, \
             tc.tile_pool(name="psagg", bufs=3, space="PSUM") as psagg:

            # --- resident state: h (fp32) and bf16 activations, layout
            # [p, nt, b, h] (node-major interleaved)
            Hsb = res.tile([128, NT, BL, H], F32, tag="Hsb")
            Hbf = res.tile([128, NT, BL, H], adt, tag="Hbf")
            Tbf = res.tile([128, NT, BL, H], adt, tag="Tbf")

            # --- constants
            wfc_hi_t = wgt.tile([IN_DIM, H], BF16, tag="wfch")
            wfc_lo_t = wgt.tile([IN_DIM, H], BF16, tag="wfcl")
            w1_t = wgt.tile([H, H], BF16, tag="w1")
            w2_t = wgt.tile([H, H], BF16, tag="w2")
            id_t = wgt.tile([128, 128], BF16, tag="ident")
            nc.sync.dma_start(wfc_hi_t[:], wfc_hi[:])
            nc.sync.dma_start(wfc_lo_t[:], wfc_lo[:])
            nc.sync.dma_start(w1_t[:], w1[:])
            nc.sync.dma_start(w2_t[:], w2[:])
            nc.sync.dma_start(id_t[:], ident[:])
            if with_bias:
                bfc_t = wgt.tile([1, H], BF16, tag="bfc")
                b1_t = wgt.tile([1, H], BF16, tag="b1")
                b2_t = wgt.tile([1, H], BF16, tag="b2")
                ones_t = wgt.tile([1, H], BF16, tag="ones")
                nc.sync.dma_start(bfc_t[:], b_fc[:])
                nc.sync.dma_start(b1_t[:], b1[:])
                nc.sync.dma_start(b2_t[:], b2[:])
                nc.sync.dma_start(ones_t[:], ones[:])

            # --- phase 0: h0 = x @ W_fc + b_fc (3-term hi/lo split)
            # chunk-outer/batch-inner so Hbf tiles complete in node order,
            # letting layer-1 aggregation start while phase 0 still streams
            for ch in range(N // XC):
                for b in range(BL):
                    xh = xs.tile([IN_DIM, XC], BF16, tag="xh")
                    xl = xs.tile([IN_DIM, XC], BF16, tag="xl")
                    nc.sync.dma_start(xh[:], xt_hi[b, :, bass.ts(ch, XC)])
                    nc.scalar.dma_start(xl[:], xt_lo[b, :, bass.ts(ch, XC)])
                    for j in range(XC // 128):
                        nt = ch * (XC // 128) + j
                        pz = ps.tile([128, H], F32, tag="pz")
                        xhs = xh[:, bass.ts(j, 128)]
                        xls = xl[:, bass.ts(j, 128)]
                        nc.tensor.matmul(pz[:], xhs, wfc_hi_t[:], start=True, stop=False)
                        nc.tensor.matmul(pz[:], xls, wfc_hi_t[:], start=False, stop=False)
                        last = not with_bias
                        nc.tensor.matmul(pz[:], xhs, wfc_lo_t[:], start=False, stop=last)
                        if with_bias:
                            nc.tensor.matmul(pz[:], ones_t[:], bfc_t[:],
                                             start=False, stop=True)
                        nc.vector.tensor_copy(Hsb[:, nt, b, :], pz[:])
                        nc.scalar.activation(Hbf[:, nt, b, :], pz[:],
                                             mybir.ActivationFunctionType.Copy)

            # --- 4 Euler steps x 2 GCN layers
            for step in range(N_STEPS):
                for layer in range(2):
                    V = Hbf if layer == 0 else Tbf
                    W = w1_t if layer == 0 else w2_t
                    bias = None if not with_bias else (b1_t if layer == 0 else b2_t)
                    for nt in range(NT):
                        blk = adjs.tile([128, NT, 128], adt, tag="adjblk")
                        nc.sync.dma_start(blk[:], adjt[nt])
                        pa = psagg.tile([128, BL, H], F32, tag="pagg")
                        if fp8:
                            for mt2 in range(NT // 2):
                                nc.tensor.matmul(
                                    pa[:], blk[:, bass.ts(mt2, 2), :],
                                    V[:, bass.ts(mt2, 2), :, :],
                                    start=(mt2 == 0), stop=(mt2 == NT // 2 - 1),
                                    perf_mode=mybir.MatmulPerfMode.DoubleRow)
                        else:
                            for mt in range(NT):
                                nc.tensor.matmul(pa[:], blk[:, mt, :], V[:, mt, :, :],
                                                 start=(mt == 0), stop=(mt == NT - 1))
                        agg = work.tile([128, BL, H], BF16, tag="agg")
                        nc.vector.tensor_copy(agg[:], pa[:])
                        # all 4 per-batch transposes into ONE psum bank tile,
                        # drained with one wide ACT copy
                        ptr = ps.tile([128, BL, 128], BF16, tag="ptr")
                        for b in range(BL):
                            nc.tensor.transpose(ptr[:, b, :], agg[:, b, :], id_t[:])
                        aggT = work.tile([128, BL, 128], BF16, tag="aggT")
                        nc.scalar.activation(aggT[:], ptr[:],
                                             mybir.ActivationFunctionType.Copy)
                        # 4 projections into ONE psum bank tile
                        pz = ps.tile([128, BL, H], F32, tag="pz")
                        for b in range(BL):
                            nc.tensor.matmul(pz[:, b, :], aggT[:, b, :], W[:],
                                             start=True, stop=bias is None)
                            if bias is not None:
                                nc.tensor.matmul(pz[:, b, :], ones_t[:], bias[:],
                                                 start=False, stop=True)
                        if layer == 0:
                            nc.scalar.activation(Tbf[:, nt, :, :], pz[:], relu)
                        else:
                            tmp = work.tile([128, BL, H], F32, tag="tmp")
                            nc.scalar.activation(tmp[:], pz[:], relu, scale=STEP)
                            nc.vector.tensor_add(Hsb[:, nt, :, :],
                                                 Hsb[:, nt, :, :], tmp[:])
                            if step == N_STEPS - 1:
                                # final h: stream out as soon as ready, on the
                                # gpsimd DMA queue so the adjT stream (sync
                                # queue) is not head-blocked
                                for b in range(BL):
                                    nc.gpsimd.dma_start(
                                        out[b, bass.ts(nt, 128), :],
                                        Hsb[:, nt, b, :])
                            else:
                                # refresh bf/fp8 copy of h in-loop (avoids a
                                # trailing cast pass at the step boundary)
                                nc.vector.tensor_copy(Hbf[:, nt, :, :],
                                                      Hsb[:, nt, :, :])

    _split_multiwait(nc)
    return nc


_NC_CACHE = {}


def _get_nc(with_bias, fp8=True):
    key = (with_bias, fp8)
    if key not in _NC_CACHE:
        _NC_CACHE[key] = _build(with_bias, fp8)
    return _NC_CACHE[key]


def _bf(a):
    return np.ascontiguousarray(a.astype(ml_dtypes.bfloat16))


def _prep_in_maps(x, adj, W_fc, b_fc, W1, b1, W2, b2, fp8=True):
    x = np.asarray(x, dtype=np.float32)
    adj = np.asarray(adj, dtype=np.float32)
    W_fc = np.asarray(W_fc, dtype=np.float32)
    b_fc = np.asarray(b_fc, dtype=np.float32)
    W1 = np.asarray(W1, dtype=np.float32)
    b1 = np.asarray(b1, dtype=np.float32)
    W2 = np.asarray(W2, dtype=np.float32)
    b2 = np.asarray(b2, dtype=np.float32)

    with_bias = bool(np.any(b_fc) or np.any(b1) or np.any(b2))

    # host layout prep (replicated operands)
    adjt = np.ascontiguousarray(
        adj.T.reshape(NT, 128, NT, 128).transpose(2, 1, 0, 3))  # [nt, p, mt, j]
    if fp8:
        adjt = np.ascontiguousarray((adjt * ADJ_SCALE).astype(ml_dtypes.float8_e4m3))
        w1h, w2h = _bf(W1 / ADJ_SCALE), _bf(W2 / ADJ_SCALE)
    else:
        adjt = _bf(adjt)
        w1h, w2h = _bf(W1), _bf(W2)
    wfc_hi = W_fc.astype(ml_dtypes.bfloat16)
    wfc_lo = _bf(W_fc - wfc_hi.astype(np.float32))
    wfc_hi = np.ascontiguousarray(wfc_hi)
    shared = {
        "adjt": adjt,
        "wfc_hi": wfc_hi, "wfc_lo": wfc_lo,
        "w1": w1h, "w2": w2h,
        "ident": np.eye(128, dtype=ml_dtypes.bfloat16),
    }
    if with_bias:
        shared.update({
            "b_fc": _bf(b_fc.reshape(1, H)),
            "b1": _bf(b1.reshape(1, H)),
            "b2": _bf(b2.reshape(1, H)),
            "ones": np.ones((1, H), dtype=ml_dtypes.bfloat16),
        })

    in_maps = []
    for c in range(N_CORES):
        xs = x[c * BL:(c + 1) * BL]                 # [BL, N, IN_DIM]
        xt = np.ascontiguousarray(xs.transpose(0, 2, 1))  # [BL, IN_DIM, N]
        xt_hi = xt.astype(ml_dtypes.bfloat16)
        xt_lo = _bf(xt - xt_hi.astype(np.float32))
        in_maps.append({**shared,
                        "xt_hi": np.ascontiguousarray(xt_hi),
                        "xt_lo": xt_lo})
    return in_maps, with_bias


FP8_DEFAULT = True


def kernel(**inputs):
    in_maps, with_bias = _prep_in_maps(**inputs, fp8=FP8_DEFAULT)
    nc = _get_nc(with_bias, FP8_DEFAULT)
    res = run_bass_kernel_spmd(nc, in_maps, core_ids=list(range(N_CORES)))
    return np.concatenate([res.results[c]["out"] for c in range(N_CORES)], axis=0)


def run_traced(**inputs):
    in_maps, with_bias = _prep_in_maps(**inputs, fp8=FP8_DEFAULT)
    nc = _get_nc(with_bias, FP8_DEFAULT)
    return run_bass_kernel_spmd(nc, in_maps, core_ids=list(range(N_CORES)),
                                trace=True)
